# revision 11
# baseline (speedup 1.0000x reference)
"""GPT-2 causal self-attention (B=2, S=2048, E=1024, H=16, D=64) on 8 TRN2 NeuronCores.

Sharding: tensor-parallel over heads - each core owns 2 heads.
  * Per core: slice of w_attn columns for its 2 heads (Q,K,V).
  * Everything is computed in a transposed layout so that no operand ever needs
    an on-chip transpose except x itself (x^T is produced once per core with PE
    transposes):
      - qT, kT stored as [d, s] (head dim on partitions) -> feed scoresT = K Q^T
      - v stored row-major [s, d] with an appended ones-column, so the
        probs@V matmul emits both ctx^T and the softmax denominator.
  * Unnormalized ctx^T (+denominators) are exchanged with a single AllToAll so
    that each core ends up with ALL heads for 1/8 of the sequence rows, then
    applies the full w_proj to its row block. No AllReduce needed.
Matmuls run in bf16 (fp32 accumulation in PSUM); scores stay fp32 in PSUM ->
exp on ScalarE (no max subtraction: scores/8 is tightly bounded for these
inputs, well within fp32 exp range).
"""

import numpy as np

import concourse.bass as bass
import concourse.mybir as mybir
import concourse.tile as tile
from concourse.bass_utils import run_bass_kernel_spmd
from concourse.masks import make_identity

B, S, E, H = 2, 2048, 1024, 16
D = E // H  # 64
NCORES = 8
HPC = H // NCORES  # 2 heads per core
R = B * S  # 4096 flattened rows
RPC = R // NCORES  # 512 output rows per core
P = 128
KO = E // P  # 8 contraction subtiles over E
QT = 512  # q tile (matmul moving free dim)
NQT = S // QT  # 4 q tiles per batch element
NKT = S // P  # 16 k tiles per batch element
NRT = R // QT  # 8 row tiles over all rows
F32 = mybir.dt.float32
BF16 = mybir.dt.bfloat16

_CACHE: dict = {}
SPLIT_WAITS = True  # sims set this False (inserted NoOps confuse CoreSim)

# ---------------------------------------------------------------------------
# This neuronxcc/walrus build rejects instructions carrying more than one
# semaphore wait ("Too many sync wait commands" in CoreV3 setupSyncWait).
# TileContext's end-of-context drain aggregates one wait per live semaphore
# onto a single Drain. Split the excess waits onto a chain of single-wait
# NOPs emitted right after the drain (all still before the tail barrier +
# semaphore reset, so semantics are unchanged).
_MAX_WAITS = 1


def _split_drain_and_barrier(self, tick_clock, wait_clock):
    from concourse.vector_clock import ScopedClock

    nc = self.nc
    drain_inst = nc.sync.drain()
    wait_clock.add_sem_waits(
        drain_inst.ins, ScopedClock({None: tick_clock.global_clock})
    )
    si = drain_inst.ins.sync_info
    waits = list(si.on_wait or [])
    if len(waits) > _MAX_WAITS:
        si.on_wait = waits[:_MAX_WAITS]
        for i in range(_MAX_WAITS, len(waits), _MAX_WAITS):
            nop = nc.sync.nop(nofuse=True, hint="drain_wait_split")
            nop.ins.sync_info = mybir.SyncInfo(
                on_wait=waits[i : i + _MAX_WAITS], on_update=[]
            )

    nc.all_engine_barrier()
    assert self.sems is not None
    popped = nc._tile_sem_poison_stack.pop()
    assert popped is self._sem_poison
    nc.clear_and_free_semaphores(list(self.sems.allocated().values()))
    nc.all_engine_barrier()


tile.TileContext._drain_and_barrier = _split_drain_and_barrier


def _split_multi_waits(nc, max_waits=1):
    """Walrus in this env rejects >1 semaphore wait per instruction. Hoist
    excess waits onto same-engine NoOps inserted immediately before the
    offending instruction (all sems are monotonic within the kernel body, so
    splitting a conjunctive wait-set across consecutive instructions on the
    same engine is semantics-preserving)."""
    n_split = 0
    for bb in nc.m.functions[0].blocks:
        out = []
        for ins in bb.instructions:
            si = ins.sync_info
            waits = list(si.on_wait) if si and si.on_wait else []
            if len(waits) > max_waits:
                extra = waits[:-max_waits]
                si.on_wait = waits[-max_waits:]
                for i in range(0, len(extra), max_waits):
                    nop = mybir.InstNoOp(
                        name=f"{ins.name}-w{i}",
                        engine=ins.engine,
                        sync_info=mybir.SyncInfo(
                            on_wait=extra[i : i + max_waits], on_update=[]
                        ),
                    )
                    out.append(nop)
                    n_split += 1
            out.append(ins)
        bb.instructions[:] = out
    return n_split


def _build():
    nc = bass.Bass(num_devices=NCORES)

    x_d = nc.declare_dram_parameter("x", [R, E], F32, isOutput=False)
    wqk_d = nc.declare_dram_parameter("wqk", [E, 2 * P], F32, isOutput=False)
    wv_d = nc.declare_dram_parameter("wv", [E, P], F32, isOutput=False)
    wp_d = nc.declare_dram_parameter("wp", [E, E], F32, isOutput=False)
    bqk_d = nc.declare_dram_parameter("bqk", [2 * P], F32, isOutput=False)
    bv_d = nc.declare_dram_parameter("bv", [P], F32, isOutput=False)
    bp_d = nc.declare_dram_parameter("bp", [E], F32, isOutput=False)
    out_d = nc.declare_dram_parameter("out_block", [RPC, E], F32, isOutput=True)

    with tile.TileContext(nc) as tc:
        with (
            tc.tile_pool(name="const", bufs=1) as const,
            tc.tile_pool(name="big", bufs=1) as big,
            tc.tile_pool(name="mm_psum", bufs=3, space="PSUM") as mm_psum,
            tc.tile_pool(name="s_psum", bufs=3, space="PSUM") as s_psum,
            tc.tile_pool(name="c_psum", bufs=2, space="PSUM") as c_psum,
            tc.tile_pool(name="dram", bufs=1, space="DRAM") as dram,
        ):
            # ---------------- constants & weights ----------------
            ident = const.tile([P, P], F32)
            make_identity(nc, ident)

            # causal masks for the 4 diagonal k-tiles of a q-tile:
            # mask[di][kp, qf] = 1.0 iff (kp + 128*di) <= qf
            masks = const.tile([P, 4, QT], BF16)
            wqk_b = const.tile([P, KO, 2 * P], BF16)
            wv_b = const.tile([P, KO, P], BF16)
            wp_b = const.tile([P, KO, E], BF16)
            bqk_s = const.tile([P, 2], F32)
            bv_s = const.tile([1, P], F32)
            bp_s = const.tile([1, E], F32)
            ones_row = const.tile([1, P], F32)
            vbias = const.tile([P, HPC, D], F32)
            bpb = const.tile([P, E], F32)
            sel = const.tile([HPC, P], BF16)

            with tc.tile_pool(name="wstage", bufs=2) as wstage:
                for di in range(4):
                    mf = wstage.tile([P, QT], F32, tag="mf")
                    nc.gpsimd.memset(mf, 1.0)
                    # keep where qf - kp - 128*di >= 0 (walrus only has is_ge)
                    nc.gpsimd.affine_select(
                        out=mf,
                        in_=mf,
                        compare_op=mybir.AluOpType.is_ge,
                        fill=0.0,
                        base=-di * P,
                        channel_multiplier=-1,
                        pattern=[[1, QT]],
                    )
                    nc.vector.tensor_copy(masks[:, di, :], mf)

                # weights: load f32 by KO-chunk, cast to bf16
                for ko in range(KO):
                    wf = wstage.tile([P, E], F32, tag="wf")
                    nc.sync.dma_start(
                        wf[:, : 2 * P], wqk_d[ko * P : (ko + 1) * P, :]
                    )
                    nc.vector.tensor_copy(wqk_b[:, ko, :], wf[:, : 2 * P])
                    wf2 = wstage.tile([P, E], F32, tag="wf")
                    nc.sync.dma_start(wf2[:, :P], wv_d[ko * P : (ko + 1) * P, :])
                    nc.vector.tensor_copy(wv_b[:, ko, :], wf2[:, :P])
                    wf3 = wstage.tile([P, E], F32, tag="wf")
                    nc.sync.dma_start(wf3, wp_d[ko * P : (ko + 1) * P, :])
                    nc.vector.tensor_copy(wp_b[:, ko, :], wf3)

                nc.sync.dma_start(bqk_s, bqk_d.rearrange("(m p) -> p m", p=P))
                nc.sync.dma_start(bv_s, bv_d[None, :])
                nc.sync.dma_start(bp_s, bp_d[None, :])
                nc.vector.memset(ones_row, 1.0)

                # broadcast b_v across partitions: [P, 128] = ones^T @ bv
                vb_ps = mm_psum.tile([P, QT], F32, tag="mm", name="vb_ps")[:, :P]
                nc.tensor.matmul(vb_ps, lhsT=ones_row, rhs=bv_s, start=True, stop=True)
                nc.vector.tensor_copy(
                    vbias, vb_ps.rearrange("p (h d) -> p h d", h=HPC)
                )

                # broadcast b_proj across partitions: [P, 1024]
                for n in range(E // QT):
                    bp_ps = mm_psum.tile([P, QT], F32, tag="mm")
                    nc.tensor.matmul(
                        bp_ps,
                        lhsT=ones_row,
                        rhs=bp_s[:, n * QT : (n + 1) * QT],
                        start=True,
                        stop=True,
                    )
                    nc.vector.tensor_copy(bpb[:, n * QT : (n + 1) * QT], bp_ps)

                # head-select matrix: sel[h, p] = 1 iff p//64 == h, built with
                # two affine_selects on iota = p - 64*h (no partition-offset
                # memsets: this walrus build rejects base_partition != 0).
                self_f = wstage.tile([HPC, P], F32, tag="sel_f")
                nc.gpsimd.memset(self_f, 1.0)
                nc.gpsimd.affine_select(
                    out=self_f, in_=self_f,
                    compare_op=mybir.AluOpType.is_ge, fill=0.0,
                    base=0, channel_multiplier=-D, pattern=[[1, P]],
                )
                nc.gpsimd.affine_select(
                    out=self_f, in_=self_f,
                    compare_op=mybir.AluOpType.is_ge, fill=0.0,
                    base=D - 1, channel_multiplier=D, pattern=[[-1, P]],
                )
                nc.vector.tensor_copy(sel, self_f)

            # ---------------- persistent activations ----------------
            qT = big.tile([P, R], BF16)  # 2 heads stacked on partitions
            kT = big.tile([P, R], BF16)
            # v row-major + ones column: [k-subtile, head, D+1]
            vsb = big.tile([P, R // P, HPC, D + 1], BF16)
            nc.vector.memset(vsb[:, :, :, D : D + 1], 1.0)

            # A2A buffers: shard j holds rows [j*512,(j+1)*512) for this core's
            # 2 heads; per shard row h*(D+1)+r: r<64 -> ctx^T, r=64 -> denom.
            a2a_in = dram.tile([NCORES, HPC * (D + 1), RPC], BF16)
            a2a_out = dram.tile([NCORES, HPC * (D + 1), RPC], BF16)

            with (
                tc.tile_pool(name="xTp", bufs=1) as xTp,
                tc.tile_pool(name="xload", bufs=6) as xload,
            ):
                xT = xTp.tile([P, KO, R], BF16)  # x^T (E on partitions)

                # ---------------- phase T: x -> x^T (bf16) ----------------
                for r0 in range(0, R, QT):
                    xt_tiles = []
                    for i in range(4):
                        x_t = xload.tile([P, E], F32, tag="x_t")
                        nc.sync.dma_start(x_t, x_d[r0 + i * P : r0 + (i + 1) * P, :])
                        xt_tiles.append(x_t)
                    for et in range(KO):
                        tp_ps = mm_psum.tile([P, QT], F32, tag="mm", name="tp_ps").rearrange(
                            "p (i q) -> p i q", i=4
                        )
                        for i in range(4):
                            nc.tensor.transpose(
                                tp_ps[:, i, :],
                                xt_tiles[i][:, et * P : (et + 1) * P],
                                ident,
                            )
                        nc.vector.tensor_copy(xT[:, et, r0 : r0 + QT], tp_ps)

                # ---------------- phase B: qT, kT, v ----------------
                for rt in range(NRT):
                    r0 = rt * QT
                    for m in range(2):  # 0 -> q cols, 1 -> k cols
                        qk_ps = mm_psum.tile([P, QT], F32, tag="mm")
                        for ko in range(KO):
                            nc.tensor.matmul(
                                qk_ps,
                                lhsT=wqk_b[:, ko, m * P : (m + 1) * P],
                                rhs=xT[:, ko, r0 : r0 + QT],
                                start=(ko == 0),
                                stop=(ko == KO - 1),
                            )
                        dst = qT if m == 0 else kT
                        nc.vector.tensor_tensor(
                            dst[:, r0 : r0 + QT],
                            qk_ps,
                            bqk_s[:, m : m + 1].to_broadcast((P, QT)),
                            mybir.AluOpType.add,
                        )
                    v_ps = mm_psum.tile([P, QT], F32, tag="mm", name="v_ps").rearrange(
                        "p (i q) -> p i q", i=4
                    )
                    for rs in range(4):
                        for ko in range(KO):
                            nc.tensor.matmul(
                                v_ps[:, rs, :],
                                lhsT=xT[:, ko, r0 + rs * P : r0 + (rs + 1) * P],
                                rhs=wv_b[:, ko, :],
                                start=(ko == 0),
                                stop=(ko == KO - 1),
                            )
                    nc.vector.tensor_tensor(
                        vsb[:, rt * 4 : (rt + 1) * 4, :, 0:D],
                        v_ps.rearrange("p r (h d) -> p r h d", h=HPC),
                        vbias[:, None, :, :].to_broadcast((P, 4, HPC, D)),
                        mybir.AluOpType.add,
                    )

            # ---------------- phase C: attention ----------------
            with (
                tc.tile_pool(name="probs", bufs=6) as probs_pool,
                tc.tile_pool(name="cstage", bufs=4) as cstage,
                tc.tile_pool(name="osb", bufs=3) as osb,
            ):
                inv_sqrt_d = 1.0 / float(np.sqrt(D))
                for b in range(B):
                    for h in range(HPC):
                        hs = slice(h * D, (h + 1) * D)
                        for qi in range(NQT):
                            q0 = b * S + qi * QT
                            nkt = 4 * (qi + 1)  # causal: only k tiles 0..nkt-1
                            ctx_ps = c_psum.tile([D + 1, QT], F32)
                            for kt in range(nkt):
                                k0 = b * S + kt * P
                                di = kt - 4 * qi
                                # causal N-trim: diagonal k-tile kt covers keys
                                # >= q0 + 128*di, so columns < delta are fully
                                # masked -> never compute/stream them.
                                delta = max(0, di) * P
                                w = QT - delta
                                sc_ps = s_psum.tile([P, QT], F32, tag="sc")
                                nc.tensor.matmul(
                                    sc_ps[:, delta:],
                                    lhsT=kT[hs, k0 : k0 + P],
                                    rhs=qT[hs, q0 + delta : q0 + QT],
                                    start=True,
                                    stop=True,
                                )
                                pr = probs_pool.tile([P, QT], BF16, tag="pr")
                                nc.scalar.activation(
                                    pr[:, delta:],
                                    sc_ps[:, delta:],
                                    mybir.ActivationFunctionType.Exp,
                                    scale=inv_sqrt_d,
                                )
                                if di >= 0:  # diagonal band: apply causal mask
                                    nc.vector.tensor_tensor(
                                        pr[:, delta:],
                                        pr[:, delta:],
                                        masks[:, di, delta:],
                                        mybir.AluOpType.mult,
                                    )
                                nc.tensor.matmul(
                                    ctx_ps[:, delta:] if delta else ctx_ps,
                                    lhsT=vsb[:, b * NKT + kt, h, :],
                                    rhs=pr[:, delta:] if delta else pr,
                                    start=(kt == 0),
                                    stop=(kt == nkt - 1),
                                )
                            ctx_sb = cstage.tile([D + 1, QT], BF16, tag="ctx_sb")
                            nc.vector.tensor_copy(ctx_sb, ctx_ps)
                            shard = b * NQT + qi  # global row block == dest core
                            nc.sync.dma_start(
                                a2a_in[shard, h * (D + 1) : (h + 1) * (D + 1), :],
                                ctx_sb,
                            )

                # ---------------- A2A ----------------
                nc.gpsimd.collective_compute(
                    "AllToAll",
                    mybir.AluOpType.bypass,
                    replica_groups=[list(range(NCORES))],
                    ins=[a2a_in[:]],
                    outs=[a2a_out[:]],
                )

                # ---------------- phase D: merge, normalize, out proj ----------------
                mT = big.tile([P, KO, RPC], BF16)  # merged^T for this core's rows
                for i in range(NCORES):
                    nc.sync.dma_start(mT[0:D, i, :], a2a_out[i, 0:D, :])
                    nc.sync.dma_start(mT[D:P, i, :], a2a_out[i, D + 1 : 2 * D + 1, :])
                # denominators: den2[h, i, q] <- a2a_out[i, 65*h + 64, q]
                den2 = big.tile([HPC, NCORES, RPC], BF16)
                nc.sync.dma_start(
                    den2,
                    a2a_out.rearrange("i (h r) q -> h i r q", h=HPC)[:, :, D, :],
                )

                denr = big.tile([P, KO, RPC], BF16)
                for i in range(NCORES):
                    db_ps = mm_psum.tile([P, QT], F32, tag="mm")
                    nc.tensor.matmul(
                        db_ps, lhsT=sel, rhs=den2[:, i, :], start=True, stop=True
                    )
                    with nc.allow_low_precision(reason="bf16 softmax denominator"):
                        nc.vector.reciprocal(denr[:, i, :], db_ps)
                nc.vector.tensor_mul(mT[:], mT[:], denr[:])

                for n in range(E // QT):
                    for ms in range(RPC // P):
                        o_ps = mm_psum.tile([P, QT], F32, tag="mm")
                        for ko in range(KO):
                            nc.tensor.matmul(
                                o_ps,
                                lhsT=mT[:, ko, ms * P : (ms + 1) * P],
                                rhs=wp_b[:, ko, n * QT : (n + 1) * QT],
                                start=(ko == 0),
                                stop=(ko == KO - 1),
                            )
                        o_sb = osb.tile([P, QT], F32, tag="o_sb")
                        nc.vector.tensor_tensor(
                            o_sb,
                            o_ps,
                            bpb[:, n * QT : (n + 1) * QT],
                            mybir.AluOpType.add,
                        )
                        nc.sync.dma_start(
                            out_d[ms * P : (ms + 1) * P, n * QT : (n + 1) * QT],
                            o_sb,
                        )

    if SPLIT_WAITS:
        _split_multi_waits(nc)
    return nc


def _get_program():
    if "nc" not in _CACHE:
        _CACHE["nc"] = _build()
    return _CACHE["nc"]


def _make_in_maps(x, w_attn, b_attn, w_proj, b_proj):
    x2 = np.ascontiguousarray(np.asarray(x, dtype=np.float32).reshape(R, E))
    w_attn = np.asarray(w_attn, dtype=np.float32)
    b_attn = np.asarray(b_attn, dtype=np.float32)
    w_proj = np.ascontiguousarray(np.asarray(w_proj, dtype=np.float32))
    b_proj = np.ascontiguousarray(np.asarray(b_proj, dtype=np.float32))

    in_maps = []
    for c in range(NCORES):
        qcols = slice(c * P, (c + 1) * P)  # heads 2c, 2c+1 of Q
        kcols = slice(E + c * P, E + (c + 1) * P)
        vcols = slice(2 * E + c * P, 2 * E + (c + 1) * P)
        wqk = np.ascontiguousarray(
            np.concatenate([w_attn[:, qcols], w_attn[:, kcols]], axis=1)
        )
        wv = np.ascontiguousarray(w_attn[:, vcols])
        bqk = np.ascontiguousarray(np.concatenate([b_attn[qcols], b_attn[kcols]]))
        bv = np.ascontiguousarray(b_attn[vcols])
        in_maps.append(
            {
                "x": x2,
                "wqk": wqk,
                "wv": wv,
                "wp": w_proj,
                "bqk": bqk,
                "bv": bv,
                "bp": b_proj,
            }
        )
    return in_maps


def _run(x, w_attn, b_attn, w_proj, b_proj):
    nc = _get_program()
    in_maps = _make_in_maps(x, w_attn, b_attn, w_proj, b_proj)
    res = run_bass_kernel_spmd(nc, in_maps, list(range(NCORES)))
    out = np.concatenate(
        [np.asarray(res.results[c]["out_block"]) for c in range(NCORES)], axis=0
    )
    return out.reshape(B, S, E).astype(np.float32), res


def kernel(x, w_attn, b_attn, w_proj, b_proj):
    out, _ = _run(x, w_attn, b_attn, w_proj, b_proj)
    return out
